# revision 22
# baseline (speedup 1.0000x reference)
"""Trainium2 Bass kernel for nn_BlockV2 (conv -> LN -> minGRU -> MLP x4).

Strategy: data-parallel over batch (B=8 -> 8 cores). Per core, activations
are kept in [D_partitions, T_free] layout and streamed through each layer in
chunks of 512 tokens; inter-layer activations ping-pong through DRAM (f32 --
the late-layer signal is a ~5e-3 variation on an O(1) baseline, so the
residual stream and all GRU gate/scan state stay fp32).

Engine balance (v2):
 - L0's depthwise conv is folded into the pointwise matmul on the host
   (P_j = diag(dw_j) @ pw, 16 accumulating matmuls per output tile) so the
   vector-bound L0 phase rides on the otherwise-idle TensorE.
 - Mid-layer depthwise conv runs on GpSimd (idle otherwise).
 - minGRU uses the z-free form: cf = sigmoid(-k), v' = (cf-1)*g(h_x) = -v,
   scan h' = cf*h' + v' with init -0.5 gives h' = -h; residual is res - h'.
 - LN rstd = (var+eps)^-1/2 via fast-inverse-sqrt bit trick + 1 Newton step
   on DVE (no Ln/Exp activation-table swaps; scalar engine stays on the
   sigmoid_and_others table set for the whole kernel).
 - LN stat matmuls on fp32 inputs use float32r (1-pass fp22) instead of
   4-pass fp32.
The minGRU recurrence runs on VectorE tensor_tensor_scan (fp32 state),
chained across chunks. Emission is a software-pipelined wavefront over
(layer-phase, chunk) as before.
"""
import sys

sys.path.insert(0, "/opt/trn_rl_repo")

from contextlib import ExitStack

import numpy as np
import ml_dtypes

import concourse.bass as bass
import concourse.tile as tile
from concourse import bacc, mybir

f32 = mybir.dt.float32
f32r = mybir.dt.float32r
bf16 = mybir.dt.bfloat16
f16 = mybir.dt.float16
i32 = mybir.dt.int32
Alu = mybir.AluOpType
Act = mybir.ActivationFunctionType
BF = ml_dtypes.bfloat16

B, D, L, K, H = 8, 512, 4, 4, 2048
N_CORES = 8
LN_EPS = 1e-5
P = 128

# fast-inverse-sqrt: y0_bits = M' - (vh_bits >> 1), with M' shifted so the
# seed approximates rsqrt(2*vh); negated seed folds in as +0x80000000.
_MAGIC = 0x5F3759DF - 0x00400000
_NEWTON_C = int(np.int32(np.uint32((_MAGIC + 1 + 0x80000000) & 0xFFFFFFFF)))


def build_nc(T=4096, CH=512, has_lnb=False):
    NCH = T // CH
    DT = D // P      # 4 d-tiles
    HT = H // P      # 16 h-tiles
    E2 = 2 * D
    MT2 = E2 // P    # 8 m-tiles of the kh matmul

    nc = bacc.Bacc("TRN2", target_bir_lowering=False, debug=False)

    xT = nc.dram_tensor("xT", [D, T + 3], bf16, kind="ExternalInput")
    pwfT = nc.dram_tensor("pwfT", [D, K, D], bf16, kind="ExternalInput")
    b0p = nc.dram_tensor("b0p", [D], f32, kind="ExternalInput")
    fwT = nc.dram_tensor("fwT", [L, D, E2], bf16, kind="ExternalInput")
    pwT = nc.dram_tensor("pwT", [L, D, D], bf16, kind="ExternalInput")
    w1T = nc.dram_tensor("w1T", [L, D, H], bf16, kind="ExternalInput")
    w2T = nc.dram_tensor("w2T", [L, H, D], bf16, kind="ExternalInput")
    dwK = nc.dram_tensor("dwK", [L, D, K], f32, kind="ExternalInput")
    dwb = nc.dram_tensor("dwb", [L, D], f32, kind="ExternalInput")
    pwb = nc.dram_tensor("pwb", [L, D], f32, kind="ExternalInput")
    b1v = nc.dram_tensor("b1v", [L, H], f32, kind="ExternalInput")
    b2v = nc.dram_tensor("b2v", [L, D], f32, kind="ExternalInput")
    lng = nc.dram_tensor("lng", [L + 1, D], f32, kind="ExternalInput")
    lnb = nc.dram_tensor("lnb", [L + 1, D], f32, kind="ExternalInput")
    out_t = nc.dram_tensor("out", [D, T], f32, kind="ExternalOutput")
    xs = [nc.dram_tensor(f"xs{i}", [D, T], f32) for i in range(2)]
    xsb = [nc.dram_tensor(f"xsb{i}", [D, T], f16) for i in range(2)]

    def dram3(tensor, c, width):
        return tensor.ap().rearrange("(dt p) t -> p dt t", p=P)[:, :, c * CH: c * CH + width]

    with tile.TileContext(nc) as tc, ExitStack() as ctx:
        sing = ctx.enter_context(tc.tile_pool(name="sing", bufs=1))
        wpool = ctx.enter_context(tc.tile_pool(name="w", bufs=1))
        big = ctx.enter_context(tc.tile_pool(name="big", bufs=5))
        small = ctx.enter_context(tc.tile_pool(name="small", bufs=6))
        gtmp = ctx.enter_context(tc.tile_pool(name="gtmp", bufs=1))
        hidp = ctx.enter_context(tc.tile_pool(name="hid", bufs=2))
        statp = ctx.enter_context(tc.tile_pool(name="stat", bufs=5))
        psmm = ctx.enter_context(tc.tile_pool(name="psmm", bufs=4, space="PSUM"))
        psst = ctx.enter_context(tc.tile_pool(name="psst", bufs=2, space="PSUM"))
        psbc = ctx.enter_context(tc.tile_pool(name="psbc", bufs=2, space="PSUM"))

        ones_col = sing.tile([P, 1], bf16)
        nc.vector.memset(ones_col, 1.0)
        ones_colf = sing.tile([P, 1], f32)
        nc.vector.memset(ones_colf, 1.0)
        ones_row = sing.tile([1, P], f32)
        nc.vector.memset(ones_row, 1.0)
        ones_row_bf = sing.tile([1, P], bf16)
        nc.vector.memset(ones_row_bf, 1.0)
        ones_col_h = sing.tile([P, 1], f16)
        nc.vector.memset(ones_col_h, 1.0)
        dw_sb = sing.tile([P, L * DT, K], f32)
        nc.sync.dma_start(out=dw_sb, in_=dwK.ap().rearrange("l (dt p) k -> p (l dt) k", p=P))
        dwb_sb = sing.tile([P, L * DT], f32)
        nc.sync.dma_start(out=dwb_sb, in_=dwb.ap().rearrange("l (dt p) -> p (l dt)", p=P))
        pwb_sb = sing.tile([P, L * DT], f32)
        nc.sync.dma_start(out=pwb_sb, in_=pwb.ap().rearrange("l (dt p) -> p (l dt)", p=P))
        b1_sb = sing.tile([P, L * HT], f32)
        nc.sync.dma_start(out=b1_sb, in_=b1v.ap().rearrange("l (ht p) -> p (l ht)", p=P))
        b2_sb = sing.tile([P, L * DT], f32)
        nc.sync.dma_start(out=b2_sb, in_=b2v.ap().rearrange("l (dt p) -> p (l dt)", p=P))
        lng_sb = sing.tile([P, (L + 1) * DT], f32)
        nc.sync.dma_start(out=lng_sb, in_=lng.ap().rearrange("l (dt p) -> p (l dt)", p=P))
        lnb_sb = sing.tile([P, (L + 1) * DT], f32)
        nc.sync.dma_start(out=lnb_sb, in_=lnb.ap().rearrange("l (dt p) -> p (l dt)", p=P))
        b0p_sb = sing.tile([P, DT], f32)
        nc.sync.dma_start(out=b0p_sb, in_=b0p.ap().rearrange("(dt p) -> p dt", p=P))
        pwf_sb = sing.tile([P, DT, K, D], bf16)
        nc.sync.dma_start(out=pwf_sb, in_=pwfT.ap().rearrange("(dt p) k e -> p dt k e", p=P))

        def load_w(kind, dram, l, shape, bufs=None):
            t = wpool.tile(shape, bf16, tag=kind, name=f"{kind}{l}", bufs=bufs)
            nc.sync.dma_start(out=t, in_=dram.ap()[l].rearrange("(kt p) e -> p kt e", p=P))
            return t

        def ln_st1(xq_tile, ones):
            """S stats from a low-precision copy of x; fp16 keeps the mu error
            off the tiny-signal gates (bf16 here costs ~1e-2 of rel err)."""
            S_ps = psst.tile([1, CH], f32, tag="ps_stat", name="S_ps")
            for kt in range(DT):
                nc.tensor.matmul(S_ps[:, :], ones[:, :], xq_tile[:, kt, :],
                                 start=(kt == 0), stop=(kt == DT - 1))
            S_sb = statp.tile([1, CH], f32, tag="stat", name="S_sb")
            nc.vector.tensor_copy(out=S_sb[:, :], in_=S_ps[:, :])
            return S_sb

        def ln_a(x_tile, S_sb):
            """broadcast mu, center in place, variance -> vh; returns (bc, vh)."""
            bc = psbc.tile([P, CH], f32, tag="ps_bc", name="bc")
            nc.tensor.matmul(bc[:, :], ones_row[:, :], S_sb[:, :], start=True, stop=True)
            for d in range(DT):
                nc.vector.scalar_tensor_tensor(
                    x_tile[:, d, :], bc[:, :], -1.0 / D, x_tile[:, d, :], Alu.mult, Alu.add)
            xsq = small.tile([P, DT, CH], bf16, tag="small", name="xsq")
            for d in range(DT):
                nc.vector.tensor_mul(xsq[:, d, :], x_tile[:, d, :], x_tile[:, d, :])
            Q_ps = psst.tile([1, CH], f32, tag="ps_stat", name="Q_ps")
            for kt in range(DT):
                nc.tensor.matmul(Q_ps[:, :], ones_col[:, :], xsq[:, kt, :],
                                 start=(kt == 0), stop=(kt == DT - 1))
            # vh = 0.5*(var + eps); rstd = 1/sqrt(2*vh) via bit-trick + Newton
            vh = statp.tile([1, CH], f32, tag="stat", name="vh")
            nc.scalar.activation(out=vh[:, :], in_=Q_ps[:, :], func=Act.Copy,
                                 bias=0.5 * LN_EPS, scale=0.5 / D)
            return bc, vh

        def ln_b(x_tile, bc, vh, slot, a_t):
            """Newton rstd on DVE, broadcast, apply into a_t."""
            seed = statp.tile([1, CH], f32, tag="stat", name="seed")
            nc.vector.tensor_scalar(out=seed[:, :].bitcast(i32), in0=vh[:, :].bitcast(i32),
                                    scalar1=1, scalar2=-1,
                                    op0=Alu.arith_shift_right, op1=Alu.bitwise_xor)
            nc.vector.tensor_scalar(out=seed[:, :].bitcast(i32), in0=seed[:, :].bitcast(i32),
                                    scalar1=_NEWTON_C, scalar2=None, op0=Alu.add)
            y2 = statp.tile([1, CH], f32, tag="stat", name="y2")
            nc.vector.tensor_mul(y2[:, :], seed[:, :], seed[:, :])
            nc.vector.tensor_mul(y2[:, :], y2[:, :], vh[:, :])
            rstd = statp.tile([1, CH], bf16, tag="stat", name="rstd")
            nc.vector.scalar_tensor_tensor(
                rstd[:, :], y2[:, :], 1.5, seed[:, :], Alu.subtract, Alu.mult)
            nc.tensor.matmul(bc[:, :], ones_row_bf[:, :], rstd[:, :], start=True, stop=True)
            for d in range(DT):
                nc.vector.scalar_tensor_tensor(
                    a_t[:, d, :], x_tile[:, d, :], lng_sb[:, slot * DT + d: slot * DT + d + 1],
                    bc[:, :], Alu.mult, Alu.mult)
            if has_lnb:
                for d in range(DT):
                    nc.vector.tensor_scalar(
                        out=a_t[:, d, :], in0=a_t[:, d, :],
                        scalar1=lnb_sb[:, slot * DT + d: slot * DT + d + 1], scalar2=None,
                        op0=Alu.add)
            return a_t

        def ln_st2(x_tile, S_sb, slot, a_t):
            bc, vh = ln_a(x_tile, S_sb)
            return ln_b(x_tile, bc, vh, slot, a_t)

        def mlp_chunk(a_t, l, w1_sb, w2_sb, out_tile, out_off):
            hid = hidp.tile([P, HT, CH], bf16, tag="hid", name="hid")
            for mt in range(HT):
                ps = psmm.tile([P, CH], f32, tag="mm", name="ps1")
                for kt in range(DT):
                    nc.tensor.matmul(ps[:, :], w1_sb[:, kt, bass.ts(mt, P)], a_t[:, kt, :],
                                     start=(kt == 0), stop=(kt == DT - 1))
                nc.scalar.activation(out=hid[:, mt, :], in_=ps[:, :], func=Act.Relu,
                                     bias=b1_sb[:, l * HT + mt: l * HT + mt + 1], scale=1.0)
            for mt in range(DT):
                ps = psmm.tile([P, CH], f32, tag="mm", name="ps2")
                for kt in range(HT):
                    nc.tensor.matmul(ps[:, :], w2_sb[:, kt, bass.ts(mt, P)], hid[:, kt, :],
                                     start=(kt == 0), stop=(kt == HT - 1))
                nc.scalar.activation(out=out_tile[:, mt, out_off: out_off + CH], in_=ps[:, :],
                                     func=Act.Identity,
                                     bias=b2_sb[:, l * DT + mt: l * DT + mt + 1], scale=1.0)

        def conv0_mm(x_in, cv0):
            """L0 conv (dw+pw folded) as 16 accumulating matmuls per out tile."""
            for mt in range(DT):
                ps = psmm.tile([P, CH], f32, tag="mm", name="ps0")
                for j in range(K):
                    for kt in range(DT):
                        nc.tensor.matmul(ps[:, :], pwf_sb[:, kt, j, bass.ts(mt, P)],
                                         x_in[:, kt, j: j + CH],
                                         start=(j == 0 and kt == 0),
                                         stop=(j == K - 1 and kt == DT - 1))
                nc.scalar.activation(out=cv0[:, mt, :], in_=ps[:, :], func=Act.Identity,
                                     bias=b0p_sb[:, mt: mt + 1], scale=1.0)
            return cv0

        def conv_dw_gp(m_t, l):
            """Depthwise conv on VectorE (Pool lacks the per-partition-scalar op)."""
            y = small.tile([P, DT, CH], bf16, tag="small", name="y")
            for d in range(DT):
                acc = gtmp.tile([P, CH], f32, tag="acc", bufs=1, name="acc")
                nc.vector.tensor_scalar(
                    out=acc, in0=m_t[:, d, 0: CH],
                    scalar1=dw_sb[:, l * DT + d, 0:1], scalar2=dwb_sb[:, l * DT + d: l * DT + d + 1],
                    op0=Alu.mult, op1=Alu.add)
                for j in range(1, K - 1):
                    nc.vector.scalar_tensor_tensor(
                        acc, m_t[:, d, j: j + CH], dw_sb[:, l * DT + d, j: j + 1],
                        acc, Alu.mult, Alu.add)
                nc.vector.scalar_tensor_tensor(
                    y[:, d, :], m_t[:, d, K - 1: K - 1 + CH], dw_sb[:, l * DT + d, K - 1: K],
                    acc, Alu.mult, Alu.add)
            return y

        def conv_pw(y, l, pw_sb, want_bf):
            cv = big.tile([P, DT, CH], f32, tag="big", name="cv")
            cv_bf = small.tile([P, DT, CH], bf16, tag="small", name="cv_bf") if want_bf else None
            for mt in range(DT):
                ps = psmm.tile([P, CH], f32, tag="mm", name="ps3")
                for kt in range(DT):
                    nc.tensor.matmul(ps[:, :], pw_sb[:, kt, bass.ts(mt, P)], y[:, kt, :],
                                     start=(kt == 0), stop=(kt == DT - 1))
                nc.scalar.activation(out=cv[:, mt, :], in_=ps[:, :], func=Act.Identity,
                                     bias=pwb_sb[:, l * DT + mt: l * DT + mt + 1], scale=1.0)
                if want_bf:
                    nc.scalar.activation(out=cv_bf[:, mt, :], in_=ps[:, :], func=Act.Identity,
                                         bias=pwb_sb[:, l * DT + mt: l * DT + mt + 1], scale=1.0)
            return cv, cv_bf

        def gru_chunk(rhs_bf, res_t, fw_sb, st, hl_tag):
            """z-free minGRU: cf=sigmoid(-k), v'=(cf-1)*g(h_x), negated scan,
            residual res -= h'. st is the prev chunk's [P, DT] last-column
            state tile (or None); h itself is stage-local so the shared hs
            rotation stays safe when phases interleave."""
            hl = gtmp.tile([P, DT], f32, tag=hl_tag, bufs=2, name="hl")
            for d in range(DT):
                # k-gate tile d, then h-gate tile d: cf is consumed immediately,
                # so its rotation stays shallow and no cross-engine WAR cycle forms.
                ps = psmm.tile([P, CH], f32, tag="mm", name="ps4")
                for kt in range(DT):
                    nc.tensor.matmul(ps[:, :], fw_sb[:, kt, bass.ts(d, P)], rhs_bf[:, kt, :],
                                     start=(kt == 0), stop=(kt == DT - 1))
                cf = gtmp.tile([P, CH], f32, tag="cf", bufs=2, name="cf")
                nc.scalar.activation(out=cf, in_=ps[:, :], func=Act.Sigmoid, scale=-1.0)
                ps2 = psmm.tile([P, CH], f32, tag="mm", name="ps5")
                for kt in range(DT):
                    nc.tensor.matmul(ps2[:, :], fw_sb[:, kt, bass.ts(DT + d, P)], rhs_bf[:, kt, :],
                                     start=(kt == 0), stop=(kt == DT - 1))
                s = gtmp.tile([P, CH], f32, tag="s", bufs=4, name="s")
                nc.scalar.activation(out=s, in_=ps2[:, :], func=Act.Sigmoid)
                nc.vector.scalar_tensor_tensor(s, ps2[:, :], 0.5, s, Alu.add, Alu.max)
                vp = gtmp.tile([P, CH], f32, tag="vp", bufs=2, name="vp")
                nc.vector.scalar_tensor_tensor(vp, cf, 1.0, s, Alu.subtract, Alu.mult)
                h = gtmp.tile([P, CH], f32, tag="hs", bufs=4, name="h")
                init = -0.5 if st is None else st[:, d: d + 1]
                nc.vector.tensor_tensor_scan(h, cf, vp, init, Alu.mult, Alu.add)
                nc.vector.tensor_copy(out=hl[:, d: d + 1], in_=h[:, CH - 1: CH])
                res_engine = nc.vector
                res_engine.tensor_sub(res_t[:, d, :], res_t[:, d, :], h)
            return hl

        # ---------- global diagonal-wavefront emission over all (layer, chunk) ----------
        l0_list = []
        mid_lists = []
        tail_list = []
        wd0 = {}
        st0 = {"h": None}

        def mk_l0(c):
            def s0(_):
                if c == 0:
                    wd0["fw"] = load_w("fw", fwT, 0, [P, DT, E2], bufs=2)
                x_in = small.tile([P, DT, CH + 3], bf16, tag="small", name="x_in")
                nc.sync.dma_start(out=x_in, in_=xT.ap().rearrange("(dt p) t -> p dt t", p=P)[:, :, c * CH: c * CH + CH + 3])
                return x_in

            def s1(x_in):
                cv0 = small.tile([P, DT, CH], bf16, tag="small", name="cv0")
                conv0_mm(x_in, cv0)
                return cv0, ln_st1(cv0, ones_col)

            def s2(art):
                cv0, S_sb = art
                bc, vh = ln_a(cv0, S_sb)
                return cv0, bc, vh

            def s3(art):
                cv0, bc, vh = art
                n = big.tile([P, DT, CH], f32, tag="big", name="n")
                ln_b(cv0, bc, vh, 0, n)
                n_bf = small.tile([P, DT, CH], bf16, tag="small", name="n_bf")
                for d in range(DT):
                    nc.scalar.activation(out=n_bf[:, d, :], in_=n[:, d, :], func=Act.Copy)
                return n, n_bf

            def s4(art):
                n, n_bf = art
                st0["h"] = gru_chunk(n_bf, n, wd0["fw"], st0["h"], "hl0")
                nb2 = small.tile([P, DT, CH], f16, tag="small", name="nb2")
                for d in range(DT):
                    nc.scalar.activation(out=nb2[:, d, :], in_=n[:, d, :], func=Act.Copy)
                nc.sync.dma_start(out=dram3(xs[0], c, CH), in_=n)
                nc.sync.dma_start(out=dram3(xsb[0], c, CH), in_=nb2)

            return [s0, s1, s2, s3, s4]

        for c in range(NCH):
            l0_list.append(mk_l0(c))

        for i in range(L - 1):
            wd = {}
            stm = {"h": None, "m_prev": None}
            src_d, dst_d = xs[i % 2], xs[(i + 1) % 2]
            src_b, dst_b = xsb[i % 2], xsb[(i + 1) % 2]
            c_w12 = 0 if i == 0 else 1
            c_fwpw = 0 if i == 0 else 3

            def mk_mid(c, i=i, wd=wd, stm=stm, src_d=src_d, dst_d=dst_d,
                       src_b=src_b, dst_b=dst_b, c_w12=c_w12, c_fwpw=c_fwpw):
                def s0(_):
                    if c == c_w12:
                        wd["w1"] = load_w("w1", w1T, i, [P, DT, H])
                        wd["w2"] = load_w("w2", w2T, i, [P, HT, D])
                    if c == c_fwpw:
                        wd["fw"] = load_w("fw", fwT, i + 1, [P, DT, E2], bufs=2)
                        wd["pw"] = load_w("pw", pwT, i + 1, [P, DT, D])
                    x_in = big.tile([P, DT, CH], f32, tag="big", name="x_in")
                    nc.sync.dma_start(out=x_in, in_=dram3(src_d, c, CH))
                    x_h = small.tile([P, DT, CH], f16, tag="small", name="x_h")
                    nc.sync.dma_start(out=x_h, in_=dram3(src_b, c, CH))
                    return (x_in, ln_st1(x_h, ones_col_h))

                def s1(art):
                    x_in, S_sb = art
                    a = small.tile([P, DT, CH], bf16, tag="small", name="a")
                    return ln_st2(x_in, S_sb, 1 + i, a)

                def s2(a):
                    m = small.tile([P, DT, CH + 3], bf16, tag="small", name="m")
                    mlp_chunk(a, i, wd["w1"], wd["w2"], m, 3)
                    if c == 0:
                        nc.vector.memset(m[:, :, 0:3], 0.0)
                    else:
                        nc.vector.tensor_copy(out=m[:, :, 0:3], in_=stm["m_prev"][:, :, CH: CH + 3])
                    stm["m_prev"] = m
                    return m

                def s3(m):
                    return conv_dw_gp(m, i + 1)

                def s4(y):
                    cv, cv_bf = conv_pw(y, i + 1, wd["pw"], want_bf=True)
                    stm["h"] = gru_chunk(cv_bf, cv, wd["fw"], stm["h"], f"hlm{i}")
                    cvb2 = small.tile([P, DT, CH], f16, tag="small", name="cvb2")
                    for d in range(DT):
                        nc.scalar.activation(out=cvb2[:, d, :], in_=cv[:, d, :], func=Act.Copy)
                    nc.sync.dma_start(out=dram3(dst_d, c, CH), in_=cv)
                    nc.sync.dma_start(out=dram3(dst_b, c, CH), in_=cvb2)

                return [s0, s1, s2, s3, s4]

            mid_lists.append([mk_mid(c) for c in range(NCH)])

        wdt = {}
        src_t = xs[(L - 1) % 2]
        src_tb = xsb[(L - 1) % 2]

        def mk_tail(c):
            def s0(_):
                if c == 1:
                    wdt["w1"] = load_w("w1", w1T, L - 1, [P, DT, H])
                    wdt["w2"] = load_w("w2", w2T, L - 1, [P, HT, D])
                x_in = big.tile([P, DT, CH], f32, tag="big", name="x_in")
                nc.sync.dma_start(out=x_in, in_=dram3(src_t, c, CH))
                x_h = small.tile([P, DT, CH], f16, tag="small", name="x_h")
                nc.sync.dma_start(out=x_h, in_=dram3(src_tb, c, CH))
                return (x_in, ln_st1(x_h, ones_col_h))

            def s1(art):
                x_in, S_sb = art
                bc, vh = ln_a(x_in, S_sb)
                return x_in, bc, vh

            def s2(art):
                x_in, bc, vh = art
                a = small.tile([P, DT, CH], bf16, tag="small", name="a")
                return ln_b(x_in, bc, vh, L, a)

            def s3(a):
                o = big.tile([P, DT, CH], f32, tag="big", name="o")
                mlp_chunk(a, L - 1, wdt["w1"], wdt["w2"], o, 0)
                nc.sync.dma_start(out=dram3(out_t, c, CH), in_=o)

            return [s0, s1, s2, s3]

        for c in range(NCH):
            tail_list.append(mk_tail(c))

        # Interleave L0's last chunks with mid0's first chunks: mid0's
        # tensor-heavy MLP work fills the vector-bound L0 window. (Only this
        # boundary -- L0 has no w1/w2 user, so no weight-slot WAR.)
        OVK = 3
        m0 = mid_lists[0]
        chunks = l0_list[:NCH - OVK]
        for j in range(OVK):
            chunks.append(l0_list[NCH - OVK + j])
            chunks.append(m0[j])
        chunks.extend(m0[OVK:])
        for ml in mid_lists[1:]:
            chunks.extend(ml)
        chunks.extend(tail_list)

        NST = 5
        arts = [None] * len(chunks)
        for g in range(len(chunks) + NST - 1):
            # oldest chunk first: every WAR target (reader of a recycled pool
            # slot) is emitted before its waiter, keeping engine queues acyclic.
            for k in reversed(range(NST)):
                idx = g - k
                if 0 <= idx < len(chunks) and k < len(chunks[idx]):
                    arts[idx] = chunks[idx][k](arts[idx])

    return nc


_CACHE = {}


def get_compiled_nc(T=4096, CH=512, has_lnb=False, **kw):
    key = (T, CH, has_lnb, tuple(sorted(kw.items())))
    if key not in _CACHE:
        nc = build_nc(T, CH, has_lnb, **kw)
        nc.compile()
        _CACHE[key] = nc
    return _CACHE[key]


def make_host_inputs(inputs, T=4096):
    f = np.float32
    dw0 = np.asarray(inputs["conv_dw_w"], f)[0]          # (K, D)
    pw0 = np.asarray(inputs["conv_pw_w"], f)[0]          # (D_out, D_in)
    # P_j[d_in, j, e] = dw0[j, d_in] * pw0[e, d_in]
    pwf = (dw0.T[:, :, None] * pw0.T[:, None, :]).astype(BF)   # (D_in, K, D_out)
    b0 = (pw0 @ np.asarray(inputs["conv_dw_b"], f)[0]
          + np.asarray(inputs["conv_pw_b"], f)[0]).astype(f)
    w = {
        "pwfT": np.ascontiguousarray(pwf),
        "b0p": b0,
        "fwT": np.ascontiguousarray(np.transpose(np.asarray(inputs["f_w"], f), (0, 2, 1))).astype(BF),
        "pwT": np.ascontiguousarray(np.transpose(np.asarray(inputs["conv_pw_w"], f), (0, 2, 1))).astype(BF),
        "w1T": np.ascontiguousarray(np.transpose(np.asarray(inputs["mlp_w1"], f), (0, 2, 1))).astype(BF),
        "w2T": np.ascontiguousarray(np.transpose(np.asarray(inputs["mlp_w2"], f), (0, 2, 1))).astype(BF),
        "dwK": np.ascontiguousarray(np.transpose(np.asarray(inputs["conv_dw_w"], f), (0, 2, 1))).astype(f),
        "dwb": np.asarray(inputs["conv_dw_b"], f),
        "pwb": np.asarray(inputs["conv_pw_b"], f),
        "b1v": np.asarray(inputs["mlp_b1"], f),
        "b2v": np.asarray(inputs["mlp_b2"], f),
        "lng": np.concatenate([np.asarray(inputs["ln1_g"], f)[None], np.asarray(inputs["ln2_g"], f)], 0),
        "lnb": np.concatenate([np.asarray(inputs["ln1_b"], f)[None], np.asarray(inputs["ln2_b"], f)], 0),
    }
    x = np.asarray(inputs["x"], f)
    nb = x.shape[0]
    in_maps = []
    for b in range(nb):
        xTp = np.zeros((D, T + 3), BF)
        xTp[:, 3:] = x[b, :T].T.astype(BF)
        in_maps.append({"xT": xTp, **w})
    has_lnb = bool(np.any(w["lnb"] != 0.0))
    return in_maps, has_lnb


def kernel(**inputs):
    from concourse.bass_utils import run_bass_kernel_spmd

    T = int(np.asarray(inputs["x"]).shape[1])
    in_maps, has_lnb = make_host_inputs(inputs, T)
    nc = get_compiled_nc(T=T, has_lnb=has_lnb)
    res = run_bass_kernel_spmd(nc, in_maps, core_ids=list(range(len(in_maps))))
    out = np.stack([r["out"].T for r in res.results])
    return np.ascontiguousarray(out.astype(np.float32))


# revision 23
# speedup vs baseline: 1.0263x; 1.0263x over previous
"""Trainium2 Bass kernel for nn_BlockV2 (conv -> LN -> minGRU -> MLP x4).

Strategy: data-parallel over batch (B=8 -> 8 cores). Per core, activations
are kept in [D_partitions, T_free] layout and streamed through each layer in
chunks of 512 tokens; inter-layer activations ping-pong through DRAM (f32 --
the late-layer signal is a ~5e-3 variation on an O(1) baseline, so the
residual stream and all GRU gate/scan state stay fp32).

Engine balance (v2):
 - L0's depthwise conv is folded into the pointwise matmul on the host
   (P_j = diag(dw_j) @ pw, 16 accumulating matmuls per output tile) so the
   vector-bound L0 phase rides on the otherwise-idle TensorE.
 - Mid-layer depthwise conv runs on GpSimd (idle otherwise).
 - minGRU uses the z-free form: cf = sigmoid(-k), v' = (cf-1)*g(h_x) = -v,
   scan h' = cf*h' + v' with init -0.5 gives h' = -h; residual is res - h'.
 - LN rstd = (var+eps)^-1/2 via fast-inverse-sqrt bit trick + 1 Newton step
   on DVE (no Ln/Exp activation-table swaps; scalar engine stays on the
   sigmoid_and_others table set for the whole kernel).
 - LN stat matmuls on fp32 inputs use float32r (1-pass fp22) instead of
   4-pass fp32.
The minGRU recurrence runs on VectorE tensor_tensor_scan (fp32 state),
chained across chunks. Emission is a software-pipelined wavefront over
(layer-phase, chunk) as before.
"""
import sys

sys.path.insert(0, "/opt/trn_rl_repo")

from contextlib import ExitStack

import numpy as np
import ml_dtypes

import concourse.bass as bass
import concourse.tile as tile
from concourse import bacc, mybir

f32 = mybir.dt.float32
f32r = mybir.dt.float32r
bf16 = mybir.dt.bfloat16
f16 = mybir.dt.float16
i32 = mybir.dt.int32
Alu = mybir.AluOpType
Act = mybir.ActivationFunctionType
BF = ml_dtypes.bfloat16

B, D, L, K, H = 8, 512, 4, 4, 2048
N_CORES = 8
LN_EPS = 1e-5
P = 128

# fast-inverse-sqrt: y0_bits = M' - (vh_bits >> 1), with M' shifted so the
# seed approximates rsqrt(2*vh); negated seed folds in as +0x80000000.
_MAGIC = 0x5F3759DF - 0x00400000
_NEWTON_C = int(np.int32(np.uint32((_MAGIC + 1 + 0x80000000) & 0xFFFFFFFF)))


def build_nc(T=4096, CH=512, has_lnb=False):
    NCH = T // CH
    DT = D // P      # 4 d-tiles
    HT = H // P      # 16 h-tiles
    E2 = 2 * D
    MT2 = E2 // P    # 8 m-tiles of the kh matmul

    nc = bacc.Bacc("TRN2", target_bir_lowering=False, debug=False)

    xT = nc.dram_tensor("xT", [D, T + 3], bf16, kind="ExternalInput")
    pwfT = nc.dram_tensor("pwfT", [D, K, D], bf16, kind="ExternalInput")
    b0p = nc.dram_tensor("b0p", [D], f32, kind="ExternalInput")
    fwT = nc.dram_tensor("fwT", [L, D, E2], bf16, kind="ExternalInput")
    pwT = nc.dram_tensor("pwT", [L, D, D], bf16, kind="ExternalInput")
    w1T = nc.dram_tensor("w1T", [L, D, H], bf16, kind="ExternalInput")
    w2T = nc.dram_tensor("w2T", [L, H, D], bf16, kind="ExternalInput")
    dwK = nc.dram_tensor("dwK", [L, D, K], f32, kind="ExternalInput")
    dwb = nc.dram_tensor("dwb", [L, D], f32, kind="ExternalInput")
    pwb = nc.dram_tensor("pwb", [L, D], f32, kind="ExternalInput")
    b1v = nc.dram_tensor("b1v", [L, H], f32, kind="ExternalInput")
    b2v = nc.dram_tensor("b2v", [L, D], f32, kind="ExternalInput")
    lng = nc.dram_tensor("lng", [L + 1, D], f32, kind="ExternalInput")
    lnb = nc.dram_tensor("lnb", [L + 1, D], f32, kind="ExternalInput")
    out_t = nc.dram_tensor("out", [D, T], f32, kind="ExternalOutput")
    xs = [nc.dram_tensor(f"xs{i}", [D, T], f32) for i in range(2)]
    xsb = [nc.dram_tensor(f"xsb{i}", [D, T], f16) for i in range(2)]

    def dram3(tensor, c, width):
        return tensor.ap().rearrange("(dt p) t -> p dt t", p=P)[:, :, c * CH: c * CH + width]

    with tile.TileContext(nc) as tc, ExitStack() as ctx:
        sing = ctx.enter_context(tc.tile_pool(name="sing", bufs=1))
        wpool = ctx.enter_context(tc.tile_pool(name="w", bufs=1))
        big = ctx.enter_context(tc.tile_pool(name="big", bufs=6))
        small = ctx.enter_context(tc.tile_pool(name="small", bufs=6))
        gtmp = ctx.enter_context(tc.tile_pool(name="gtmp", bufs=1))
        hidp = ctx.enter_context(tc.tile_pool(name="hid", bufs=2))
        statp = ctx.enter_context(tc.tile_pool(name="stat", bufs=5))
        psmm = ctx.enter_context(tc.tile_pool(name="psmm", bufs=4, space="PSUM"))
        psst = ctx.enter_context(tc.tile_pool(name="psst", bufs=2, space="PSUM"))
        psbc = ctx.enter_context(tc.tile_pool(name="psbc", bufs=2, space="PSUM"))

        ones_col = sing.tile([P, 1], bf16)
        nc.vector.memset(ones_col, 1.0)
        ones_colf = sing.tile([P, 1], f32)
        nc.vector.memset(ones_colf, 1.0)
        ones_row = sing.tile([1, P], f32)
        nc.vector.memset(ones_row, 1.0)
        ones_row_bf = sing.tile([1, P], bf16)
        nc.vector.memset(ones_row_bf, 1.0)
        ones_col_h = sing.tile([P, 1], f16)
        nc.vector.memset(ones_col_h, 1.0)
        dw_sb = sing.tile([P, L * DT, K], f32)
        nc.sync.dma_start(out=dw_sb, in_=dwK.ap().rearrange("l (dt p) k -> p (l dt) k", p=P))
        dwb_sb = sing.tile([P, L * DT], f32)
        nc.sync.dma_start(out=dwb_sb, in_=dwb.ap().rearrange("l (dt p) -> p (l dt)", p=P))
        pwb_sb = sing.tile([P, L * DT], f32)
        nc.sync.dma_start(out=pwb_sb, in_=pwb.ap().rearrange("l (dt p) -> p (l dt)", p=P))
        b1_sb = sing.tile([P, L * HT], f32)
        nc.sync.dma_start(out=b1_sb, in_=b1v.ap().rearrange("l (ht p) -> p (l ht)", p=P))
        b2_sb = sing.tile([P, L * DT], f32)
        nc.sync.dma_start(out=b2_sb, in_=b2v.ap().rearrange("l (dt p) -> p (l dt)", p=P))
        lng_sb = sing.tile([P, (L + 1) * DT], f32)
        nc.sync.dma_start(out=lng_sb, in_=lng.ap().rearrange("l (dt p) -> p (l dt)", p=P))
        lnb_sb = sing.tile([P, (L + 1) * DT], f32)
        nc.sync.dma_start(out=lnb_sb, in_=lnb.ap().rearrange("l (dt p) -> p (l dt)", p=P))
        b0p_sb = sing.tile([P, DT], f32)
        nc.sync.dma_start(out=b0p_sb, in_=b0p.ap().rearrange("(dt p) -> p dt", p=P))
        pwf_sb = sing.tile([P, DT, K, D], bf16)
        nc.sync.dma_start(out=pwf_sb, in_=pwfT.ap().rearrange("(dt p) k e -> p dt k e", p=P))

        def load_w(kind, dram, l, shape, bufs=None):
            t = wpool.tile(shape, bf16, tag=kind, name=f"{kind}{l}", bufs=bufs)
            nc.sync.dma_start(out=t, in_=dram.ap()[l].rearrange("(kt p) e -> p kt e", p=P))
            return t

        def ln_st1(xq_tile, ones):
            """S stats from a low-precision copy of x; fp16 keeps the mu error
            off the tiny-signal gates (bf16 here costs ~1e-2 of rel err)."""
            S_ps = psst.tile([1, CH], f32, tag="ps_stat", name="S_ps")
            for kt in range(DT):
                nc.tensor.matmul(S_ps[:, :], ones[:, :], xq_tile[:, kt, :],
                                 start=(kt == 0), stop=(kt == DT - 1))
            S_sb = statp.tile([1, CH], f32, tag="stat", name="S_sb")
            nc.vector.tensor_copy(out=S_sb[:, :], in_=S_ps[:, :])
            return S_sb

        def ln_a(x_tile, S_sb):
            """broadcast mu, center in place, variance -> vh; returns (bc, vh)."""
            bc = psbc.tile([P, CH], f32, tag="ps_bc", name="bc")
            nc.tensor.matmul(bc[:, :], ones_row[:, :], S_sb[:, :], start=True, stop=True)
            for d in range(DT):
                nc.vector.scalar_tensor_tensor(
                    x_tile[:, d, :], bc[:, :], -1.0 / D, x_tile[:, d, :], Alu.mult, Alu.add)
            xsq = small.tile([P, DT, CH], bf16, tag="small", name="xsq")
            for d in range(DT):
                nc.scalar.activation(out=xsq[:, d, :], in_=x_tile[:, d, :], func=Act.Square)
            Q_ps = psst.tile([1, CH], f32, tag="ps_stat", name="Q_ps")
            for kt in range(DT):
                nc.tensor.matmul(Q_ps[:, :], ones_col[:, :], xsq[:, kt, :],
                                 start=(kt == 0), stop=(kt == DT - 1))
            # vh = 0.5*(var + eps); rstd = 1/sqrt(2*vh) via bit-trick + Newton
            vh = statp.tile([1, CH], f32, tag="stat", name="vh")
            nc.scalar.activation(out=vh[:, :], in_=Q_ps[:, :], func=Act.Copy,
                                 bias=0.5 * LN_EPS, scale=0.5 / D)
            return bc, vh

        def ln_b(x_tile, bc, vh, slot, a_t):
            """Newton rstd on DVE, broadcast, apply into a_t."""
            seed = statp.tile([1, CH], f32, tag="stat", name="seed")
            nc.vector.tensor_scalar(out=seed[:, :].bitcast(i32), in0=vh[:, :].bitcast(i32),
                                    scalar1=1, scalar2=-1,
                                    op0=Alu.arith_shift_right, op1=Alu.bitwise_xor)
            nc.vector.tensor_scalar(out=seed[:, :].bitcast(i32), in0=seed[:, :].bitcast(i32),
                                    scalar1=_NEWTON_C, scalar2=None, op0=Alu.add)
            y2 = statp.tile([1, CH], f32, tag="stat", name="y2")
            nc.vector.tensor_mul(y2[:, :], seed[:, :], seed[:, :])
            nc.vector.tensor_mul(y2[:, :], y2[:, :], vh[:, :])
            rstd = statp.tile([1, CH], bf16, tag="stat", name="rstd")
            nc.vector.scalar_tensor_tensor(
                rstd[:, :], y2[:, :], 1.5, seed[:, :], Alu.subtract, Alu.mult)
            nc.tensor.matmul(bc[:, :], ones_row_bf[:, :], rstd[:, :], start=True, stop=True)
            for d in range(DT):
                nc.vector.scalar_tensor_tensor(
                    a_t[:, d, :], x_tile[:, d, :], lng_sb[:, slot * DT + d: slot * DT + d + 1],
                    bc[:, :], Alu.mult, Alu.mult)
            if has_lnb:
                for d in range(DT):
                    nc.vector.tensor_scalar(
                        out=a_t[:, d, :], in0=a_t[:, d, :],
                        scalar1=lnb_sb[:, slot * DT + d: slot * DT + d + 1], scalar2=None,
                        op0=Alu.add)
            return a_t

        def ln_st2(x_tile, S_sb, slot, a_t):
            bc, vh = ln_a(x_tile, S_sb)
            return ln_b(x_tile, bc, vh, slot, a_t)

        def mlp_chunk(a_t, l, w1_sb, w2_sb, out_tile, out_off):
            hid = hidp.tile([P, HT, CH], bf16, tag="hid", name="hid")
            for mt in range(HT):
                ps = psmm.tile([P, CH], f32, tag="mm", name="ps1")
                for kt in range(DT):
                    nc.tensor.matmul(ps[:, :], w1_sb[:, kt, bass.ts(mt, P)], a_t[:, kt, :],
                                     start=(kt == 0), stop=(kt == DT - 1))
                nc.scalar.activation(out=hid[:, mt, :], in_=ps[:, :], func=Act.Relu,
                                     bias=b1_sb[:, l * HT + mt: l * HT + mt + 1], scale=1.0)
            for mt in range(DT):
                ps = psmm.tile([P, CH], f32, tag="mm", name="ps2")
                for kt in range(HT):
                    nc.tensor.matmul(ps[:, :], w2_sb[:, kt, bass.ts(mt, P)], hid[:, kt, :],
                                     start=(kt == 0), stop=(kt == HT - 1))
                nc.scalar.activation(out=out_tile[:, mt, out_off: out_off + CH], in_=ps[:, :],
                                     func=Act.Identity,
                                     bias=b2_sb[:, l * DT + mt: l * DT + mt + 1], scale=1.0)

        def conv0_mm(x_in, cv0):
            """L0 conv (dw+pw folded) as 16 accumulating matmuls per out tile."""
            for mt in range(DT):
                ps = psmm.tile([P, CH], f32, tag="mm", name="ps0")
                for j in range(K):
                    for kt in range(DT):
                        nc.tensor.matmul(ps[:, :], pwf_sb[:, kt, j, bass.ts(mt, P)],
                                         x_in[:, kt, j: j + CH],
                                         start=(j == 0 and kt == 0),
                                         stop=(j == K - 1 and kt == DT - 1))
                nc.scalar.activation(out=cv0[:, mt, :], in_=ps[:, :], func=Act.Identity,
                                     bias=b0p_sb[:, mt: mt + 1], scale=1.0)
            return cv0

        def conv_dw_gp(m_t, l):
            """Depthwise conv on VectorE (Pool lacks the per-partition-scalar op)."""
            y = small.tile([P, DT, CH], bf16, tag="small", name="y")
            for d in range(DT):
                acc = gtmp.tile([P, CH], f32, tag="acc", bufs=1, name="acc")
                nc.vector.tensor_scalar(
                    out=acc, in0=m_t[:, d, 0: CH],
                    scalar1=dw_sb[:, l * DT + d, 0:1], scalar2=dwb_sb[:, l * DT + d: l * DT + d + 1],
                    op0=Alu.mult, op1=Alu.add)
                for j in range(1, K - 1):
                    nc.vector.scalar_tensor_tensor(
                        acc, m_t[:, d, j: j + CH], dw_sb[:, l * DT + d, j: j + 1],
                        acc, Alu.mult, Alu.add)
                nc.vector.scalar_tensor_tensor(
                    y[:, d, :], m_t[:, d, K - 1: K - 1 + CH], dw_sb[:, l * DT + d, K - 1: K],
                    acc, Alu.mult, Alu.add)
            return y

        def conv_pw(y, l, pw_sb, want_bf):
            cv = big.tile([P, DT, CH], f32, tag="big", name="cv")
            cv_bf = small.tile([P, DT, CH], bf16, tag="small", name="cv_bf") if want_bf else None
            for mt in range(DT):
                ps = psmm.tile([P, CH], f32, tag="mm", name="ps3")
                for kt in range(DT):
                    nc.tensor.matmul(ps[:, :], pw_sb[:, kt, bass.ts(mt, P)], y[:, kt, :],
                                     start=(kt == 0), stop=(kt == DT - 1))
                nc.scalar.activation(out=cv[:, mt, :], in_=ps[:, :], func=Act.Identity,
                                     bias=pwb_sb[:, l * DT + mt: l * DT + mt + 1], scale=1.0)
                if want_bf:
                    nc.scalar.activation(out=cv_bf[:, mt, :], in_=ps[:, :], func=Act.Identity,
                                         bias=pwb_sb[:, l * DT + mt: l * DT + mt + 1], scale=1.0)
            return cv, cv_bf

        def gru_chunk(rhs_bf, res_t, fw_sb, st, hl_tag):
            """z-free minGRU: cf=sigmoid(-k), v'=(cf-1)*g(h_x), negated scan,
            residual res -= h'. st is the prev chunk's [P, DT] last-column
            state tile (or None); h itself is stage-local so the shared hs
            rotation stays safe when phases interleave."""
            hl = gtmp.tile([P, DT], f32, tag=hl_tag, bufs=2, name="hl")
            for d in range(DT):
                # k-gate tile d, then h-gate tile d: cf is consumed immediately,
                # so its rotation stays shallow and no cross-engine WAR cycle forms.
                ps = psmm.tile([P, CH], f32, tag="mm", name="ps4")
                for kt in range(DT):
                    nc.tensor.matmul(ps[:, :], fw_sb[:, kt, bass.ts(d, P)], rhs_bf[:, kt, :],
                                     start=(kt == 0), stop=(kt == DT - 1))
                cf = gtmp.tile([P, CH], f32, tag="cf", bufs=2, name="cf")
                nc.scalar.activation(out=cf, in_=ps[:, :], func=Act.Sigmoid, scale=-1.0)
                ps2 = psmm.tile([P, CH], f32, tag="mm", name="ps5")
                for kt in range(DT):
                    nc.tensor.matmul(ps2[:, :], fw_sb[:, kt, bass.ts(DT + d, P)], rhs_bf[:, kt, :],
                                     start=(kt == 0), stop=(kt == DT - 1))
                s = gtmp.tile([P, CH], f32, tag="s", bufs=4, name="s")
                nc.scalar.activation(out=s, in_=ps2[:, :], func=Act.Sigmoid)
                nc.vector.scalar_tensor_tensor(s, ps2[:, :], 0.5, s, Alu.add, Alu.max)
                vp = gtmp.tile([P, CH], f32, tag="vp", bufs=2, name="vp")
                nc.vector.scalar_tensor_tensor(vp, cf, 1.0, s, Alu.subtract, Alu.mult)
                h = gtmp.tile([P, CH], f32, tag="hs", bufs=4, name="h")
                init = -0.5 if st is None else st[:, d: d + 1]
                nc.vector.tensor_tensor_scan(h, cf, vp, init, Alu.mult, Alu.add)
                nc.vector.tensor_copy(out=hl[:, d: d + 1], in_=h[:, CH - 1: CH])
                res_engine = nc.vector
                res_engine.tensor_sub(res_t[:, d, :], res_t[:, d, :], h)
            return hl

        # ---------- global diagonal-wavefront emission over all (layer, chunk) ----------
        l0_list = []
        mid_lists = []
        tail_list = []
        wd0 = {}
        st0 = {"h": None}

        def mk_l0(c):
            def s0(_):
                if c == 0:
                    wd0["fw"] = load_w("fw", fwT, 0, [P, DT, E2])
                x_in = small.tile([P, DT, CH + 3], bf16, tag="small", name="x_in")
                nc.sync.dma_start(out=x_in, in_=xT.ap().rearrange("(dt p) t -> p dt t", p=P)[:, :, c * CH: c * CH + CH + 3])
                return x_in

            def s1(x_in):
                cv0 = small.tile([P, DT, CH], bf16, tag="small", name="cv0")
                conv0_mm(x_in, cv0)
                return cv0, ln_st1(cv0, ones_col)

            def s2(art):
                cv0, S_sb = art
                bc, vh = ln_a(cv0, S_sb)
                return cv0, bc, vh

            def s3(art):
                cv0, bc, vh = art
                n = big.tile([P, DT, CH], f32, tag="big", name="n")
                ln_b(cv0, bc, vh, 0, n)
                n_bf = small.tile([P, DT, CH], bf16, tag="small", name="n_bf")
                for d in range(DT):
                    nc.scalar.activation(out=n_bf[:, d, :], in_=n[:, d, :], func=Act.Copy)
                return n, n_bf

            def s4(art):
                n, n_bf = art
                st0["h"] = gru_chunk(n_bf, n, wd0["fw"], st0["h"], "hl0")
                nb2 = small.tile([P, DT, CH], f16, tag="small", name="nb2")
                for d in range(DT):
                    nc.scalar.activation(out=nb2[:, d, :], in_=n[:, d, :], func=Act.Copy)
                nc.sync.dma_start(out=dram3(xs[0], c, CH), in_=n)
                nc.sync.dma_start(out=dram3(xsb[0], c, CH), in_=nb2)

            return [s0, s1, s2, s3, s4]

        for c in range(NCH):
            l0_list.append(mk_l0(c))

        for i in range(L - 1):
            wd = {}
            stm = {"h": None, "m_prev": None}
            src_d, dst_d = xs[i % 2], xs[(i + 1) % 2]
            src_b, dst_b = xsb[i % 2], xsb[(i + 1) % 2]
            c_w12 = 0 if i == 0 else 1
            c_fwpw = 3

            def mk_mid(c, i=i, wd=wd, stm=stm, src_d=src_d, dst_d=dst_d,
                       src_b=src_b, dst_b=dst_b, c_w12=c_w12, c_fwpw=c_fwpw):
                def s0(_):
                    if c == c_w12:
                        wd["w1"] = load_w("w1", w1T, i, [P, DT, H])
                        wd["w2"] = load_w("w2", w2T, i, [P, HT, D])
                    if c == c_fwpw:
                        wd["fw"] = load_w("fw", fwT, i + 1, [P, DT, E2])
                        wd["pw"] = load_w("pw", pwT, i + 1, [P, DT, D])
                    x_in = big.tile([P, DT, CH], f32, tag="big", name="x_in")
                    nc.sync.dma_start(out=x_in, in_=dram3(src_d, c, CH))
                    x_h = small.tile([P, DT, CH], f16, tag="small", name="x_h")
                    nc.sync.dma_start(out=x_h, in_=dram3(src_b, c, CH))
                    return (x_in, ln_st1(x_h, ones_col_h))

                def s1(art):
                    x_in, S_sb = art
                    a = small.tile([P, DT, CH], bf16, tag="small", name="a")
                    return ln_st2(x_in, S_sb, 1 + i, a)

                def s2(a):
                    m = small.tile([P, DT, CH + 3], bf16, tag="small", name="m")
                    mlp_chunk(a, i, wd["w1"], wd["w2"], m, 3)
                    if c == 0:
                        nc.vector.memset(m[:, :, 0:3], 0.0)
                    else:
                        nc.scalar.activation(out=m[:, :, 0:3], in_=stm["m_prev"][:, :, CH: CH + 3], func=Act.Copy)
                    stm["m_prev"] = m
                    return m

                def s3(m):
                    return conv_dw_gp(m, i + 1)

                def s4(y):
                    cv, cv_bf = conv_pw(y, i + 1, wd["pw"], want_bf=True)
                    stm["h"] = gru_chunk(cv_bf, cv, wd["fw"], stm["h"], f"hlm{i}")
                    cvb2 = small.tile([P, DT, CH], f16, tag="small", name="cvb2")
                    for d in range(DT):
                        nc.scalar.activation(out=cvb2[:, d, :], in_=cv[:, d, :], func=Act.Copy)
                    nc.sync.dma_start(out=dram3(dst_d, c, CH), in_=cv)
                    nc.sync.dma_start(out=dram3(dst_b, c, CH), in_=cvb2)

                return [s0, s1, s2, s3, s4]

            mid_lists.append([mk_mid(c) for c in range(NCH)])

        wdt = {}
        src_t = xs[(L - 1) % 2]
        src_tb = xsb[(L - 1) % 2]

        def mk_tail(c):
            def s0(_):
                if c == 1:
                    wdt["w1"] = load_w("w1", w1T, L - 1, [P, DT, H])
                    wdt["w2"] = load_w("w2", w2T, L - 1, [P, HT, D])
                x_in = big.tile([P, DT, CH], f32, tag="big", name="x_in")
                nc.sync.dma_start(out=x_in, in_=dram3(src_t, c, CH))
                x_h = small.tile([P, DT, CH], f16, tag="small", name="x_h")
                nc.sync.dma_start(out=x_h, in_=dram3(src_tb, c, CH))
                return (x_in, ln_st1(x_h, ones_col_h))

            def s1(art):
                x_in, S_sb = art
                bc, vh = ln_a(x_in, S_sb)
                return x_in, bc, vh

            def s2(art):
                x_in, bc, vh = art
                a = small.tile([P, DT, CH], bf16, tag="small", name="a")
                return ln_b(x_in, bc, vh, L, a)

            def s3(a):
                o = big.tile([P, DT, CH], f32, tag="big", name="o")
                mlp_chunk(a, L - 1, wdt["w1"], wdt["w2"], o, 0)
                nc.sync.dma_start(out=dram3(out_t, c, CH), in_=o)

            return [s0, s1, s2, s3]

        for c in range(NCH):
            tail_list.append(mk_tail(c))

        chunks = l0_list + [c for ml in mid_lists for c in ml] + tail_list

        NST = 5
        arts = [None] * len(chunks)
        for g in range(len(chunks) + NST - 1):
            # oldest chunk first: every WAR target (reader of a recycled pool
            # slot) is emitted before its waiter, keeping engine queues acyclic.
            for k in reversed(range(NST)):
                idx = g - k
                if 0 <= idx < len(chunks) and k < len(chunks[idx]):
                    arts[idx] = chunks[idx][k](arts[idx])

    return nc


_CACHE = {}


def get_compiled_nc(T=4096, CH=512, has_lnb=False, **kw):
    key = (T, CH, has_lnb, tuple(sorted(kw.items())))
    if key not in _CACHE:
        nc = build_nc(T, CH, has_lnb, **kw)
        nc.compile()
        _CACHE[key] = nc
    return _CACHE[key]


def make_host_inputs(inputs, T=4096):
    f = np.float32
    dw0 = np.asarray(inputs["conv_dw_w"], f)[0]          # (K, D)
    pw0 = np.asarray(inputs["conv_pw_w"], f)[0]          # (D_out, D_in)
    # P_j[d_in, j, e] = dw0[j, d_in] * pw0[e, d_in]
    pwf = (dw0.T[:, :, None] * pw0.T[:, None, :]).astype(BF)   # (D_in, K, D_out)
    b0 = (pw0 @ np.asarray(inputs["conv_dw_b"], f)[0]
          + np.asarray(inputs["conv_pw_b"], f)[0]).astype(f)
    w = {
        "pwfT": np.ascontiguousarray(pwf),
        "b0p": b0,
        "fwT": np.ascontiguousarray(np.transpose(np.asarray(inputs["f_w"], f), (0, 2, 1))).astype(BF),
        "pwT": np.ascontiguousarray(np.transpose(np.asarray(inputs["conv_pw_w"], f), (0, 2, 1))).astype(BF),
        "w1T": np.ascontiguousarray(np.transpose(np.asarray(inputs["mlp_w1"], f), (0, 2, 1))).astype(BF),
        "w2T": np.ascontiguousarray(np.transpose(np.asarray(inputs["mlp_w2"], f), (0, 2, 1))).astype(BF),
        "dwK": np.ascontiguousarray(np.transpose(np.asarray(inputs["conv_dw_w"], f), (0, 2, 1))).astype(f),
        "dwb": np.asarray(inputs["conv_dw_b"], f),
        "pwb": np.asarray(inputs["conv_pw_b"], f),
        "b1v": np.asarray(inputs["mlp_b1"], f),
        "b2v": np.asarray(inputs["mlp_b2"], f),
        "lng": np.concatenate([np.asarray(inputs["ln1_g"], f)[None], np.asarray(inputs["ln2_g"], f)], 0),
        "lnb": np.concatenate([np.asarray(inputs["ln1_b"], f)[None], np.asarray(inputs["ln2_b"], f)], 0),
    }
    x = np.asarray(inputs["x"], f)
    nb = x.shape[0]
    in_maps = []
    for b in range(nb):
        xTp = np.zeros((D, T + 3), BF)
        xTp[:, 3:] = x[b, :T].T.astype(BF)
        in_maps.append({"xT": xTp, **w})
    has_lnb = bool(np.any(w["lnb"] != 0.0))
    return in_maps, has_lnb


def kernel(**inputs):
    from concourse.bass_utils import run_bass_kernel_spmd

    T = int(np.asarray(inputs["x"]).shape[1])
    in_maps, has_lnb = make_host_inputs(inputs, T)
    nc = get_compiled_nc(T=T, has_lnb=has_lnb)
    res = run_bass_kernel_spmd(nc, in_maps, core_ids=list(range(len(in_maps))))
    out = np.stack([r["out"].T for r in res.results])
    return np.ascontiguousarray(out.astype(np.float32))
